# revision 72
# baseline (speedup 1.0000x reference)
"""Trainium2 Bass kernel for DDN depth-focal loss (nn_DDNLoss).

Data-parallel over batch B=8 across 8 NeuronCores (1 image per core).
Each core computes sum_px(weight * focal(depth_logits, target)) for its
image; the host sums the 8 partials and divides by B*H*W.

Algorithm (device, per core):
  1. Rasterize the z-buffer with ONE bf16 matmul: box n gets key
     8^(1+slot_n) where slot_n indexes the image's distinct target bins
     sorted DESCENDING (nearest box == smallest bin == largest slot ==
     largest key).  PSUM accumulates sum_n key_n * rowmask_n x colmask_n.
     Because bin(depth) is monotone, per-pixel max-key == reference's
     min-depth painter result; and floor(log8) of the SUM recovers the
     top slot exactly (m<=5 boxes per bin certified on host).
  2. Decode slot = trunc((biased_exp(S) - 130)/3) via int bitcast; bounce
     (96,320) -> (128,240) pixel-major through DRAM.  The bounce DMA is
     dispatched between chunk DMAs so it lands early in the DMA FIFO.
  3. Stream logits (fp16, 6 DMA chunks); Act exp per strip; channel-sum
     per strip via in-place fp16 pairwise-fold tree on DVE (2x mode).
  4. Gather x[target] from the RAW logits via per-slot int16 masks
     (Pool) + copy_predicated (DVE) in two column passes overlapped
     with the logit DMA; slots split into two accumulators (gat/gat_b)
     to halve the RAW-chain depth, merged by one predicated copy.
  5. Focal epilogue: pt = exp(x_t)*recip(S); logp = x_t - ln(S);
     loss = (1-pt)^2*logp*(-alpha*w); Ln is issued before the final exq
     so its table load hides under the gather tail; the 128 per-partition
     partials are DMA'd out and summed on host with the 8 core partials.
"""

import os

import numpy as np
import ml_dtypes

import concourse.bacc as bacc
import concourse.mybir as mybir
from concourse import bass_isa, tile
from concourse.bass_utils import run_bass_kernel_spmd

# Problem constants (hardcoded per harness contract).
B, C, H, W, N = 8, 81, 96, 320, 32
CP = 82                 # channels padded to even
HW = H * W              # 30720 pixels per image
P = 128                 # partitions
J = HW // P             # 240 pixel columns per partition (partition-major)
NSTRIP = 8
JS = J // NSTRIP        # 30 pixel columns per strip
FS = JS * CP            # strip free size
NCHUNK = 4              # DMA chunks (2 strips each)

ALPHA = 0.25
FG_W, BG_W = 13.0, 1.0
DEPTH_MIN, DEPTH_MAX, NUM_BINS = 0.001, 60.0, 80
BIN_SIZE = 2.0 * (DEPTH_MAX - DEPTH_MIN) / (NUM_BINS * (1 + NUM_BINS))
PAD_LOGIT = -20.0       # exp(-20) ~ 2e-9: invisible in the softmax sum

F32 = mybir.dt.float32
BF16 = mybir.dt.bfloat16
FP16 = mybir.dt.float16
I32 = mybir.dt.int32
I16 = mybir.dt.int16
U8 = mybir.dt.uint8
Alu = mybir.AluOpType
Act = mybir.ActivationFunctionType

_CACHE = {}
LAST_RESULTS = None


def _fold_widths():
    """In-place pairwise fold schedule 82 -> 1 (pad col 81 is zero-ish)."""
    steps = [(0, 18, 64, 82)]  # fold [64:82) into [0:18)
    w = 64
    while w > 1:
        h = w // 2
        steps.append((0, h, h, w))
        w = h
    return steps


# DMA chunking (strips per chunk): first chunks small to cut pipeline
# latency; the direct SBUF->SBUF bounce DMA is dispatched between chunk 2
# and chunk 3 so it lands early in the DMA-engine FIFO.
CHUNKS = [1, 1, 1, 1, 2, 2]
BOUNCE_AFTER = 2                  # bounce dispatched after this many chunks
# gather pass col ranges [j0, j1); pass p is gated by the chunk DMA that
# covers its columns.  Each pass is emitted as two interleaved half-column
# chains so consecutive copy_predicated ops hit independent RAW chains.
CP_PASSES = [(0, 90), (90, 240)]
G_POOL = 0                        # Pool gather route disabled (STT not supported on Pool)


def _build(nslot, stage=5):
    nc = bacc.Bacc("TRN2", target_bir_lowering=False, debug=False)

    xt = nc.dram_tensor("xt", [P, J * CP], FP16, kind="ExternalInput")
    rkcm = nc.dram_tensor("rkcm", [N, H + W], BF16, kind="ExternalInput")
    outv = nc.dram_tensor("outv", [P, 1], F32, kind="ExternalOutput")

    ndve = nslot - G_POOL         # slots 0..ndve-1 on DVE, rest on Pool

    with tile.TileContext(nc) as tc:
        with (
            tc.tile_pool(name="big", bufs=1) as bigp,
            tc.tile_pool(name="est", bufs=4) as estp,
            tc.tile_pool(name="map", bufs=1) as mapp,
            tc.tile_pool(name="msk", bufs=1) as mskp,
            tc.tile_pool(name="sml", bufs=1) as smlp,
            tc.tile_pool(name="drp", bufs=1, space="DRAM") as drp,
            tc.tile_pool(name="ps", bufs=2, space="PSUM") as psp,
        ):
            with nc.allow_low_precision(reason="fp16 softmax path, tol 2e-2"):
                # ---- inputs: raster operands in one DMA, then xt chunks;
                # the bounce DMA is emitted mid-stream (FIFO position) ----
                rkcm_t = smlp.tile([N, H + W], BF16)
                nc.sync.dma_start(rkcm_t[:], rkcm[:])

                xs = bigp.tile([P, J * CP], FP16)
                xs3 = xs[:].rearrange("p (j c) -> p j c", c=CP)
                posr = mapp.tile([P, J], I16)

                # first chunks dispatch ahead of the bounce
                strip0 = 0
                for nstr in CHUNKS[:BOUNCE_AFTER]:
                    sl = slice(strip0 * FS, (strip0 + nstr) * FS)
                    nc.sync.dma_start(xs[:, sl], xt[:, sl])
                    strip0 += nstr

                zs = psp.tile([H, W], F32, tag="zs")
                nc.tensor.matmul(
                    zs[:], rkcm_t[:, 0:H], rkcm_t[:, H : H + W], start=True, stop=True
                )

                # ---- decode slot index from the f32 exponent (DVE, early):
                # slot = trunc((biased_exp - 130)/3) via int bitcast ----
                zf = smlp.tile([H, W], F32)
                nc.vector.tensor_copy(zf[:], zs[:])
                posf = smlp.tile([H, W], F32)
                nc.gpsimd.tensor_scalar(
                    posf[:], zf[:].bitcast(I32), float(130 * (1 << 23)),
                    1.0 / (3 * (1 << 23)), Alu.subtract, Alu.mult,
                )
                posh = smlp.tile([H, W], I16)
                nc.gpsimd.tensor_copy(posh[:], posf[:])

                # relayout (96,320) -> (128,240) via a DRAM bounce (linear
                # pixel order both ways); SP blocks here so the remaining
                # chunk transfers queue after the bounce in the DMA FIFO
                pbounce = drp.tile([1, HW], I16)
                nc.sync.dma_start(pbounce[:], posh[:])
                nc.sync.dma_start(posr[:], pbounce[:])

                # remaining chunks
                for nstr in CHUNKS[BOUNCE_AFTER:]:
                    sl = slice(strip0 * FS, (strip0 + nstr) * FS)
                    nc.sync.dma_start(xs[:, sl], xt[:, sl])
                    strip0 += nstr

                def probe(src):
                    pr = smlp.tile([1, 1], F32, name="probe")
                    nc.vector.tensor_copy(pr[:], src)
                    nc.sync.dma_start(outv[:], pr[:])

                if stage == 1:
                    probe(posr[0:1, 0:1])

                # ---- Pool stream: masks for DVE slots, pool-route gather,
                # pmask/weights ----
                K_DVE_MASK = 6
                masks = []
                if stage >= 2:
                    for s in range(ndve):
                        mk = mskp.tile([P, J], I16, name=f"mk{s}", tag=f"mk{s}")
                        if s >= K_DVE_MASK:
                            nc.gpsimd.tensor_scalar(
                                mk[:], posr[:], float(s), None, Alu.is_equal
                            )
                        masks.append(mk)
                    for s in range(min(K_DVE_MASK, ndve)):
                        nc.vector.tensor_scalar(
                            masks[s][:], posr[:], float(s), None, Alu.is_equal
                        )

                gat = mapp.tile([P, J], FP16)

                if stage >= 2:
                    # weights pre-scaled by -ALPHA: {-0.25, -3.25} (exact fp16)
                    wt16 = mapp.tile([P, J], FP16)
                    nc.gpsimd.tensor_scalar(
                        wt16[:], posr[:], 0.0, -3.0, Alu.is_ge, Alu.mult
                    )
                    nc.gpsimd.tensor_scalar(wt16[:], wt16[:], -0.25, None, Alu.add)
                    if stage == 2:
                        probe(wt16[0:1, 0:1])

                # ---- DVE stream: trees + gather passes, interleaved in
                # approximate data-readiness order ----
                sred16 = mapp.tile([P, J], FP16)
                folds = _fold_widths()
                es_tiles = {}

                def emit_exp(st):
                    ssl = slice(st * FS, (st + 1) * FS)
                    es = estp.tile([P, FS], FP16, name=f"es{st}", tag="es")
                    nc.scalar.activation(es[:], xs[:, ssl], Act.Exp)
                    es_tiles[st] = es

                def tree_ops(st):
                    e3 = es_tiles[st][:].rearrange("p (j c) -> p j c", c=CP)
                    ops = []
                    for (d0, d1, s0, s1) in folds[:-1]:
                        ops.append((e3[:, :, d0:d1], e3[:, :, d0:d1], e3[:, :, s0:s1]))
                    ops.append((
                        sred16[:, st * JS : (st + 1) * JS].rearrange(
                            "p (j o) -> p j o", o=1
                        ),
                        e3[:, :, 0:1], e3[:, :, 1:2],
                    ))
                    return ops

                half_s = (ndve + 1) // 2
                gat_b = mapp.tile([P, J], FP16)
                gmask = mapp.tile([P, J], I16)
                nc.gpsimd.tensor_scalar(
                    gmask[:], posr[:], float(half_s), None, Alu.is_ge
                )

                def cp_ops(j0, j1, gen_masks=False):
                    # two independent RAW chains (slot groups), interleaved,
                    # then a single merge cp
                    ops = [("init", j0, j1)]
                    for k in range(half_s):
                        ops.append(("cp", j0, j1, k))
                        if half_s + k < ndve:
                            ops.append(("cp", j0, j1, half_s + k))
                    ops.append(("merge", j0, j1))
                    return ops

                def emit_dve(op):
                    if isinstance(op, tuple) and op and op[0] == "mask":
                        _, s = op
                        nc.vector.tensor_scalar(
                            masks[s][:], posr[:], float(s), None, Alu.is_equal
                        )
                    elif isinstance(op, tuple) and op and op[0] == "init":
                        _, j0, j1 = op
                        nc.vector.tensor_copy(gat[:, j0:j1], xs3[:, j0:j1, 0:1])
                    elif isinstance(op, tuple) and op and op[0] == "cp":
                        _, j0, j1, s = op
                        dst = gat if s < half_s else gat_b
                        nc.vector.copy_predicated(
                            dst[:, j0:j1], masks[s][:, j0:j1],
                            xs3[:, j0:j1, 1 + s : 2 + s],
                        )
                    elif isinstance(op, tuple) and op and op[0] == "merge":
                        _, j0, j1 = op
                        nc.vector.copy_predicated(
                            gat[:, j0:j1], gmask[:, j0:j1], gat_b[:, j0:j1]
                        )
                    else:
                        o, a, b = op
                        nc.vector.tensor_tensor(o, a, b, Alu.add)

                def interleave(*lists):
                    """Merge op lists round-robin proportionally to length."""
                    lists = [list(l) for l in lists if l]
                    total = sum(len(l) for l in lists)
                    out = []
                    idx = [0] * len(lists)
                    for k in range(total):
                        best, bestv = 0, 1e9
                        for i, l in enumerate(lists):
                            if idx[i] < len(l):
                                v = idx[i] / len(l)
                                if v < bestv:
                                    best, bestv = i, v
                        out.append(lists[best][idx[best]])
                        idx[best] += 1
                    return out

                exq = mapp.tile([P, J], FP16)
                lns = mapp.tile([P, J], FP16)

                gatd = mapp.tile([P, J], FP16)

                def emit_merge_exq(j0, j1):
                    if j1 >= J:
                        # Ln first: its table load runs in Act's idle window
                        # right after the exps; gatd's fake dep on lns keeps
                        # the scheduler from hoisting exq (and a third table
                        # load) ahead of it.
                        nc.scalar.activation(lns[:], sred16[:], Act.Ln)
                        nc.vector.scalar_tensor_tensor(
                            gatd[:], lns[:], 0.0, gat[:], Alu.mult, Alu.add
                        )
                        nc.scalar.activation(exq[:], gatd[:], Act.Exp)

                do_cp = stage >= 4
                # Act stream: exps in strip order (emitted as we go)
                if stage >= 3:
                    emit_exp(0)
                    emit_exp(1)
                    emit_exp(2)
                    emit_exp(3)
                    # cp pass 1 (cols 0-90, strips 0-2) with trees 0-3;
                    # the first slots' masks are generated inline on DVE so
                    # the gather is not paced by Pool's mask stream
                    for op in interleave(
                        cp_ops(*CP_PASSES[0], gen_masks=True) if do_cp else [],
                        tree_ops(0), tree_ops(1), tree_ops(2), tree_ops(3),
                    ):
                        emit_dve(op)
                    if do_cp:
                        emit_merge_exq(*CP_PASSES[0])
                    emit_exp(4)
                    emit_exp(5)
                    emit_exp(6)
                    emit_exp(7)
                    # cp pass 2 (cols 90-240, strips 3-7) with trees 4-7
                    for op in interleave(
                        cp_ops(*CP_PASSES[1]) if do_cp else [],
                        tree_ops(4), tree_ops(5), tree_ops(6), tree_ops(7),
                    ):
                        emit_dve(op)
                    if do_cp:
                        emit_merge_exq(*CP_PASSES[1])
                    if stage == 3:
                        probe(sred16[0:1, 0:1])
                    elif stage == 4:
                        probe(gat[0:1, 0:1])

                if stage >= 5:
                    # ---- softmax pieces ----
                    rs32 = mapp.tile([P, J], F32)
                    nc.vector.reciprocal(rs32[:], sred16[:])
                    rs16 = mapp.tile([P, J], FP16)
                    nc.vector.tensor_copy(rs16[:], rs32[:])

                    # ---- focal epilogue (single engine, no sync pingpong) ----
                    pt = mapp.tile([P, J], FP16)
                    nc.vector.tensor_tensor(pt[:], exq[:], rs16[:], Alu.mult)
                    um = mapp.tile([P, J], FP16)
                    nc.vector.tensor_scalar(um[:], pt[:], -1.0, 1.0, Alu.mult, Alu.add)
                    um2 = mapp.tile([P, J], FP16)
                    nc.vector.tensor_tensor(um2[:], um[:], um[:], Alu.mult)
                    logp = mapp.tile([P, J], FP16)
                    nc.vector.tensor_tensor(logp[:], gat[:], lns[:], Alu.subtract)
                    wl = mapp.tile([P, J], FP16)
                    nc.vector.tensor_tensor(wl[:], logp[:], wt16[:], Alu.mult)
                    junk = mapp.tile([P, J], FP16)
                    nc.vector.tensor_tensor(junk[:], um2[:], wl[:], Alu.mult)
                    acc = mapp.tile([P, 1], F32)
                    nc.vector.tensor_reduce(
                        acc[:], junk[:], axis=mybir.AxisListType.X, op=Alu.add
                    )
                    nc.sync.dma_start(outv[:], acc[:])

    nc.finalize()
    return nc


def _ref_bin_vec(d):
    """Target bins replicating the reference's float32 LID binning."""
    d = d.astype(np.float32)
    a = np.float32(1.0) + np.float32(8.0) * (d - np.float32(DEPTH_MIN)) / np.float32(
        BIN_SIZE
    )
    idx = np.float32(-0.5) + np.float32(0.5) * np.sqrt(a, dtype=np.float32)
    return idx.astype(np.int32)  # trunc toward zero; idx >= 0 here


def _host_prep(depth_logits, gt_boxes2d, gt_center_depth):
    """Stage per-core inputs: slot-keyed raster masks + permuted fp16 logits."""
    xt = np.transpose(depth_logits, (0, 2, 3, 1)).reshape(B, HW, C)
    boxes = gt_boxes2d.reshape(B, N, 4)
    depths = gt_center_depth.reshape(B, N)

    fbox = np.concatenate(
        [np.floor(boxes[:, :, :2]), np.ceil(boxes[:, :, 2:])], axis=2
    )
    bins = _ref_bin_vec(depths)  # (B, N)

    uu = np.arange(W, dtype=np.float64)
    vv = np.arange(H, dtype=np.float64)

    xtp = np.empty((B, P, J * CP), np.float16)
    rkcm = np.zeros((B, N, H + W), ml_dtypes.bfloat16)
    nslots = []
    for b in range(B):
        dist = sorted(set(bins[b].tolist()), reverse=True)  # bins DESC
        slot_of = {bn: i for i, bn in enumerate(dist)}
        nslots.append(len(dist))

        cnt = {bn: int((bins[b] == bn).sum()) for bn in dist}
        if max(cnt.values()) > 5:
            # Exact decode certificate: simulate the f32 psum sum and check
            # the exponent decode against the true z-buffer winner.
            S = np.zeros((H, W), np.float32)
            for n in range(N):
                key = np.float32(8.0 ** (1 + slot_of[bins[b, n]]))
                rm = ((vv >= fbox[b, n, 1]) & (vv < fbox[b, n, 3])).astype(np.float32)
                cmk = ((uu >= fbox[b, n, 0]) & (uu < fbox[b, n, 2])).astype(np.float32)
                S = S + key * np.outer(rm, cmk)
            dec = np.trunc(
                (S.view(np.int32).astype(np.float64) - 130 * 2**23) / (3 * 2**23)
            ).astype(np.int32)
            cand = np.where(
                (vv[None, :, None] >= fbox[b, :, 1, None, None])
                & (vv[None, :, None] < fbox[b, :, 3, None, None])
                & (uu[None, None, :] >= fbox[b, :, 0, None, None])
                & (uu[None, None, :] < fbox[b, :, 2, None, None]),
                depths[b][:, None, None],
                np.inf,
            )
            cov = np.isfinite(cand).any(0)
            want = np.full((H, W), -1, np.int32)
            wbin = _ref_bin_vec(cand.min(0)[cov].astype(np.float32))
            want[cov] = np.array([slot_of[int(x)] for x in wbin], np.int32)
            got = np.where(dec >= 0, dec, -1)
            if not (got == want).all():
                raise RuntimeError(f"slot-key decode mismatch on image {b}")

        for n in range(N):
            key = np.float32(8.0 ** (1 + slot_of[bins[b, n]]))
            rm = (vv >= fbox[b, n, 1]) & (vv < fbox[b, n, 3])
            cmk = (uu >= fbox[b, n, 0]) & (uu < fbox[b, n, 2])
            rkcm[b, n, :H] = (rm.astype(np.float32) * key).astype(ml_dtypes.bfloat16)
            rkcm[b, n, H:] = cmk.astype(np.float32).astype(ml_dtypes.bfloat16)

        # channel permutation: col 0 = bg(80); col 1+s = slot-s bin; leftovers
        perm = [NUM_BINS] + dist + [
            ch for ch in range(C) if ch != NUM_BINS and ch not in slot_of
        ]
        assert len(perm) == C
        xb = np.full((HW, CP), PAD_LOGIT, np.float16)
        xb[:, :C] = xt[b][:, perm].astype(np.float16)
        xtp[b] = xb.reshape(P, J * CP)

    return xtp, rkcm, max(nslots)


def kernel(depth_logits, gt_boxes2d, gt_boxes3d, gt_center_depth, num_gt_per_img):
    depth_logits = np.asarray(depth_logits, dtype=np.float32)
    gt_boxes2d = np.asarray(gt_boxes2d, dtype=np.float32)
    gt_center_depth = np.asarray(gt_center_depth, dtype=np.float32)

    xtp, rkcm, nslot = _host_prep(depth_logits, gt_boxes2d, gt_center_depth)

    if nslot not in _CACHE:
        _CACHE[nslot] = _build(nslot)
    nc = _CACHE[nslot]

    in_maps = []
    for b in range(B):
        in_maps.append(
            {
                "xt": np.ascontiguousarray(xtp[b]),
                "rkcm": np.ascontiguousarray(rkcm[b]),
            }
        )

    res = run_bass_kernel_spmd(
        nc,
        in_maps,
        core_ids=list(range(B)),
        trace=bool(os.environ.get("DDN_TRACE")),
    )
    global LAST_RESULTS
    LAST_RESULTS = res
    total = 0.0
    for b in range(B):
        total += float(res.results[b]["outv"].sum(dtype=np.float64))
    return np.float32(total / (B * H * W))
